# revision 1
# baseline (speedup 1.0000x reference)
"""NT-Xent loss on 8 Trainium2 NeuronCores (Bass/Tile).

Math
----
reference: rows = interleave(zjs, zis) [2B, D]; zn = rows/max(|row|,eps);
S = (zn @ zn.T)/0.5; mask diag; loss = -mean_i log_softmax(S)[i, pair(i)].

The loss is invariant to any joint row/column permutation, so we use the
STACKED order rows = [zjs; zis] with pair(i) = i +- B.  Since every score
is 2*cos <= 2 and the diagonal exp(2*cos_ii - 2) == 1 (+/- fp eps), no
masking or row-max pass is needed:

    lse_i  = 2 + ln( sum_j exp(2 cos_ij - 2) - 1 )
    loss   = 2 + ( sum_i ln(rowsum_i - 1) - 2 * sum_i cos_{i,pair(i)} ) / 2B

Distribution: each core gets the full transposed rep matrix [D, 2B]
(= "all-gathered Zn" state), ROLLED along columns by c*1024 so that the
uniform SPMD program always treats columns [0:1024] as its local row
block and [4096:5120] as the positive partners.  Each core normalizes
the full matrix (cheap), computes its 1024x8192 block of the similarity
matrix in bf16 on the TensorEngine, fuses exp+rowsum on the Scalar
engine (accum_out), and emits two partial sums; the host combines 8
pairs of scalars.

Scheduling: engines execute instructions in program order, so emission
hand-pipelines per-engine streams (normalize chunk c interleaved with
main-phase columns of earlier chunks).  Inverse norms are computed as
Exp(-0.5*Ln(ss)); with the ACT table set pinned to
natural_log_exp_and_others, the kernel performs zero table reloads.

Host-side work is layout-only (concat/transpose/roll/replicate): all
arithmetic (normalization, matmul, softmax, log, reductions) is on
device.
"""

import numpy as np
from contextlib import ExitStack

import concourse.bass as bass
import concourse.tile as tile
from concourse import bacc, mybir
from concourse.bass_utils import run_bass_kernel_spmd
from concourse._compat import with_exitstack

B = 4096
D = 256
N = 2 * B                 # 8192 rows/cols of the similarity matrix
N_CORES = 8
LOCAL = N // N_CORES      # 1024 rows per core
CHUNK = 2048              # normalize / matmul-group column chunk
NCHUNK = N // CHUNK       # 4
KC = D // 128             # 2 contraction chunks of 128
MTILES = LOCAL // 128     # 8 m-tiles of 128 rows
F32 = mybir.dt.float32
BF16 = mybir.dt.bfloat16
AF = mybir.ActivationFunctionType
X = mybir.AxisListType.X

@with_exitstack
def _ntxent_kernel(ctx: ExitStack, tc: tile.TileContext, rt_ap, out_ap):
    nc = tc.nc

    sb_rt = ctx.enter_context(tc.tile_pool(name="rt", bufs=2 * KC * NCHUNK))
    sb_sq = ctx.enter_context(tc.tile_pool(name="sq", bufs=4))
    sb_inv = ctx.enter_context(tc.tile_pool(name="inv", bufs=2))
    sb_znt = ctx.enter_context(tc.tile_pool(name="znt", bufs=1))
    sb_dmy = ctx.enter_context(tc.tile_pool(name="dmy", bufs=2))
    sb_fin = ctx.enter_context(tc.tile_pool(name="fin", bufs=1))
    ps = ctx.enter_context(tc.tile_pool(name="ps", bufs=2, space="PSUM"))

    # constants
    ones128 = sb_fin.tile([128, 1], F32, tag="ones128")
    nc.vector.memset(ones128[:], 1.0)
    onesb = sb_fin.tile([128, 128], BF16, tag="onesb")
    nc.vector.memset(onesb[:], 1.0)
    neg2 = sb_fin.tile([128, 1], F32, tag="neg2")
    nc.vector.memset(neg2[:], -2.0)
    neg1 = sb_fin.tile([128, 1], F32, tag="neg1")
    nc.vector.memset(neg1[:], -1.0)

    # persistent tiles
    znt = sb_znt.tile([128, KC, N], BF16)            # normalized, transposed reps
    racc = sb_fin.tile([128, MTILES * NCHUNK], F32, tag="racc")
    lgacc = sb_fin.tile([128, MTILES], F32, tag="lgacc")
    pacc = sb_fin.tile([128, KC], F32, tag="pacc")

    # all input DMAs issued up-front
    rtk = {}
    for c in range(NCHUNK):
        for k in range(KC):
            t = sb_rt.tile([128, CHUNK], F32, tag="rt")
            nc.sync.dma_start(out=t[:], in_=rt_ap[k][:, bass.ds(c * CHUNK, CHUNK)])
            rtk[(c, k)] = t

    def squares(c):
        sqs = []
        for k in range(KC):
            sq = sb_sq.tile([128, CHUNK], BF16, tag="sq")
            nc.vector.tensor_mul(sq[:], rtk[(c, k)][:], rtk[(c, k)][:])
            sqs.append(sq)
        return sqs

    def colsum(c, sqs):
        # column sums of squares, broadcast across partitions via the
        # ones[128,128] stationary operand
        ns2b = ps.tile([128, CHUNK], F32, tag="ps")
        for k in range(KC):
            for j in range(CHUNK // 512):
                jsl = bass.ds(j * 512, 512)
                nc.tensor.matmul(ns2b[:, jsl], onesb[:], sqs[k][:, jsl],
                                 start=(k == 0), stop=(k == KC - 1))
        return ns2b

    def invnorm(c, ns2b):
        # 1/sqrt(ss) = exp(-0.5*ln(ss)); both functions live in the
        # pinned natural_log_exp table set, so no table reloads.
        lnb = sb_inv.tile([128, CHUNK], F32, tag="inv")
        nc.scalar.activation(lnb[:], ns2b[:], AF.Ln, bias=0.0, scale=1.0)
        invb = sb_inv.tile([128, CHUNK], F32, tag="inv")
        nc.scalar.activation(invb[:], lnb[:], AF.Exp, bias=0.0, scale=-0.5)
        return invb

    def apply(c, invb):
        csl = bass.ds(c * CHUNK, CHUNK)
        for k in range(KC):
            nc.vector.tensor_mul(znt[:, k, csl], rtk[(c, k)][:], invb[:])

    def main_column(g, ms):
        # 128x2048 similarity blocks for m-tiles `ms` against chunk g
        for m in ms:
            msl = bass.ds(m * 128, 128)
            pst = ps.tile([128, CHUNK], F32, tag="ps")
            for k in range(KC):
                for j in range(CHUNK // 512):
                    jsl = bass.ds(j * 512, 512)
                    nc.tensor.matmul(pst[:, jsl], znt[:, k, msl],
                                     znt[:, k, bass.ds(g * CHUNK + j * 512, 512)],
                                     start=(k == 0), stop=(k == KC - 1))
            dmy = sb_dmy.tile([128, CHUNK], BF16, tag="dmy")
            nc.scalar.activation(dmy[:], pst[:], AF.Exp, bias=neg2[:], scale=2.0,
                                 accum_out=racc[:, bass.ds(m * NCHUNK + g, 1)])

    def pos_phase():
        # positive-pair cosines for local rows (cols 0:1024 x 4096:5120)
        for k in range(KC):
            pprod = sb_dmy.tile([128, LOCAL], BF16, tag="dmy")
            nc.vector.tensor_mul(pprod[:], znt[:, k, 0:LOCAL],
                                 znt[:, k, bass.ds(B, LOCAL)])
            nc.vector.reduce_sum(pacc[:, bass.ds(k, 1)], pprod[:], axis=X)

    # Pin the ACT table set to natural_log_exp_and_others (id 6): it
    # contains every function this kernel uses (Ln, Exp, Copy), so
    # walrus's lower_act inserts no further table loads.  Without this,
    # Ln and Exp resolve to two different sets and every invnorm pair
    # costs two ~1.3us reloads.
    nc.scalar.add_instruction(mybir.InstLoadActFuncSet(
        name=nc.get_next_instruction_name(), ins=[], outs=[],
        act_func_set_id=6))


    # Hand-pipelined emission. Engines execute their streams in program
    # order, so each engine's stream must avoid waiting on work that a
    # LATER entry of another engine's stream produces.
    #   DVE: sq0 sq1 app0 sq2 app1 sq3 app2 pos app3
    #   PE : cs0 cs1 g0[0:4] cs2 g0[4:] g1[0:4] cs3 g1[4:] g2 g3
    #   ACT: inv0 inv1 e(g0,0:4) inv2 e(g0,4:) e(g1,0:4) inv3 e(g1,4:) ...
    front = list(range(MTILES // 2))
    back = list(range(MTILES // 2, MTILES))
    sq0 = squares(0)
    sq1 = squares(1)
    ns0 = colsum(0, sq0)
    ns1 = colsum(1, sq1)
    iv0 = invnorm(0, ns0)
    apply(0, iv0)
    sq2 = squares(2)
    iv1 = invnorm(1, ns1)
    apply(1, iv1)
    main_column(0, front)
    ns2 = colsum(2, sq2)
    iv2 = invnorm(2, ns2)
    sq3 = squares(3)
    apply(2, iv2)
    main_column(0, back)
    main_column(1, front)
    ns3 = colsum(3, sq3)
    iv3 = invnorm(3, ns3)
    apply(3, iv3)
    pos_phase()
    main_column(1, back)
    main_column(2, range(MTILES))
    main_column(3, range(MTILES))

    # rowsums -> ln(rowsum - 1) in one op per phase
    rsall = sb_fin.tile([128, MTILES], F32, tag="rsall")
    nc.vector.reduce_sum(
        rsall[:], racc[:].rearrange("p (m g) -> p m g", g=NCHUNK), axis=X)
    nc.scalar.activation(lgacc[:], rsall[:], AF.Ln, bias=neg1[:], scale=1.0)

    # ---- fold to two scalars ----
    fin = sb_fin.tile([128, 2], F32, tag="fin")
    nc.vector.reduce_sum(fin[:, 0:1], lgacc[:], axis=X)
    nc.vector.reduce_sum(fin[:, 1:2], pacc[:], axis=X)
    psf = ps.tile([1, 2], F32, tag="ps")
    nc.tensor.matmul(psf[:], ones128[:], fin[:], start=True, stop=True)
    ob = sb_fin.tile([1, 2], F32, tag="ob")
    nc.scalar.copy(ob[:], psf[:])
    nc.sync.dma_start(out=out_ap[:, :], in_=ob[:])


_NC_CACHE = None


def _build_program():
    global _NC_CACHE
    if _NC_CACHE is not None:
        return _NC_CACHE
    nc = bacc.Bacc("TRN2", target_bir_lowering=False, debug=False,
                   num_devices=N_CORES)
    rt = nc.dram_tensor("rt", [KC, 128, N], F32, kind="ExternalInput").ap()
    out = nc.dram_tensor("out", [1, 2], F32, kind="ExternalOutput").ap()
    with tile.TileContext(nc) as tc:
        _ntxent_kernel(tc, rt, out)
    nc.finalize()
    _NC_CACHE = nc
    return nc


def kernel(zis: np.ndarray, zjs: np.ndarray) -> np.ndarray:
    assert zis.shape == (B, D) and zjs.shape == (B, D)
    nc = _build_program()

    # Host prep (layout only): stack, transpose to [D, N], split the
    # contraction dim, and roll columns so each core's local block is
    # at a uniform offset.
    rt_full = np.ascontiguousarray(
        np.concatenate([zjs, zis], axis=0).T.astype(np.float32, copy=False)
    ).reshape(KC, 128, N)

    in_maps = []
    for c in range(N_CORES):
        rolled = np.roll(rt_full, -c * LOCAL, axis=2)
        in_maps.append({"rt": np.ascontiguousarray(rolled)})

    res = run_bass_kernel_spmd(nc, in_maps, core_ids=list(range(N_CORES)))

    log_sum = 0.0
    pos_sum = 0.0
    for c in range(N_CORES):
        o = res.results[c]["out"]
        log_sum += float(o[0, 0])
        pos_sum += float(o[0, 1])
    loss = 2.0 + (log_sum - 2.0 * pos_sum) / N
    return np.asarray(loss, dtype=np.float32)



# revision 23
# speedup vs baseline: 2.0245x; 2.0245x over previous
"""NT-Xent loss on 8 Trainium2 NeuronCores (Bass/Tile).

Math
----
reference: rows = interleave(zjs, zis) [2B, D]; zn = rows/max(|row|,eps);
S = (zn @ zn.T)/0.5; mask diag; loss = -mean_i log_softmax(S)[i, pair(i)].

The loss is invariant to any joint row/column permutation, so we use the
STACKED order rows = [zjs; zis] with pair(i) = i +- B.  Every score is
2*cos <= 2 and the diagonal exp(2*cos_ii - 2) == 1, so no masking or
row-max pass is needed:

    lse_i  = 2 + ln( sum_j exp(2 cos_ij - 2) - 1 )
    loss   = 2 + ( sum_i ln(rowsum_i - 1) - 2 * sum_i cos_{i,pair(i)} ) / 2B

Kernel strategy (per core; inputs rolled by c*1024 columns so columns
[0:1024] are the local row block, [4096:5120] the positives):

* similarity blocks are computed TRANSPOSED: one unit = [128 global
  cols x 1024 local rows], via fp8e4m3 DoubleRow matmuls (both k-tiles
  per instruction, 0.5 cyc/row).  The stationary side is RAW quantized
  z8; only the 1024 local (moving) columns are pre-normalized.  The
  missing 1/|col| factor rides into the exp as a per-partition scale.
* exp(2 cos - 2) must read PSUM, which only ACT and DVE can (GPSIMD
  cannot): ACT units use the hw Exp table; DVE units use a Schraudolph
  bit hack (t = x*8/ln2 + const, truncate to int8, bitcast fp8e4m3),
  tuned so the rowsum bias is ~1e-4.  All exp outputs are fp8.
* per-row sums of exp become PARTITION-axis sums on the Tensor engine:
  ones-stationary DoubleRow matmuls (16-wide stationary: dual-fp8
  ldweights rejects narrower weights) accumulate into a [16, 1024]
  PSUM tile, one bank per row-half (~107ns per 128x512 block).
* Pool (GPSIMD) owns all SBUF-side support: squares, the local
  normalize-multiply, positive-pair products and the scale vectors.
  Column norms: compact sq colsums via tiny matmuls -> ACT Ln/Exp on
  [128, 8]; the local broadcast invnorm uses a quake-style rsqrt bit
  hack on DVE (no PSUM->SBUF round trip through ACT).

Host-side work is layout + dtype quantization only; all arithmetic is
on device.  Per-core output is [1, 3]: two ln-rowsum partials and the
positive-cosine partial; the host sums 8x3 scalars.
"""

import numpy as np
import ml_dtypes
from contextlib import ExitStack

import concourse.bass as bass
import concourse.tile as tile
from concourse import bacc, mybir
from concourse.bass_utils import run_bass_kernel_spmd
from concourse._compat import with_exitstack

B = 4096
D = 256
N = 2 * B                  # 8192 rows/cols of the similarity matrix
N_CORES = 8
LOCAL = N // N_CORES       # 1024 local rows per core
KC = D // 128              # 2 contraction k-tiles
NT = N // 128              # 64 column tiles of 128
HALF = 512
SQC = 1024                 # squares chunk (columns)
NSQ = N // SQC             # 8 squares chunks
F32 = mybir.dt.float32
BF16 = mybir.dt.bfloat16
FP8 = mybir.dt.float8e4
I8 = mybir.dt.int8
I16 = mybir.dt.int16
AF = mybir.ActivationFunctionType
DR = mybir.MatmulPerfMode.DoubleRow
X = mybir.AxisListType.X
ALU = mybir.AluOpType

E4 = ml_dtypes.float8_e4m3
BF = ml_dtypes.bfloat16

# Schraudolph fp8e4m3 exp: bits = trunc(x * 8/ln2 + SC2); the -0.4 bias
# is tuned so the mean approx/exact ratio over the cos distribution ~ 1.
A8 = 8.0 / np.log(2.0)
SC2 = 56.0 - 2.0 * A8 + 0.5 - 0.4
K16 = 0x5F38   # quake-rsqrt magic for bf16 bits (tuned end-to-end)

# engine per half-tile: "A" (ACT exp) or "D" (DVE schraudolph).
QUOTA = {"A": 65, "D": 63}
D_FROM = 4     # first half-tile eligible for DVE


def _mk_assign():
    acc = {"A": 0.0, "D": 0.0}
    left = dict(QUOTA)
    out = []
    for t in range(2 * NT):
        for e in ("A", "D"):
            if left[e] > 0 and (e != "D" or t >= D_FROM):
                acc[e] += QUOTA[e] / (2 * NT if e == "A" else 2 * NT - D_FROM)
        cand = [e for e in ("A", "D") if left[e] > 0
                and (e != "D" or t >= D_FROM)]
        e = max(cand, key=lambda k: acc[k])
        acc[e] -= 1.0
        left[e] -= 1
        out.append(e)
    return out


ASSIGN = _mk_assign()


@with_exitstack
def _ntxent_kernel(ctx: ExitStack, tc: tile.TileContext, z8_ap, zb_ap, out_ap):
    nc = tc.nc

    sb = ctx.enter_context(tc.tile_pool(name="sb", bufs=1))
    sbsq = ctx.enter_context(tc.tile_pool(name="sq", bufs=NSQ))
    sbe = ctx.enter_context(tc.tile_pool(name="e8", bufs=6))
    sbt = ctx.enter_context(tc.tile_pool(name="tmp", bufs=2))
    ps = ctx.enter_context(tc.tile_pool(name="ps", bufs=5, space="PSUM"))
    psr = ctx.enter_context(tc.tile_pool(name="psr", bufs=1, space="PSUM"))

    # ---- constants ----
    onesb = sb.tile([128, 128], BF16, tag="onesb")
    nc.vector.memset(onesb[:], 1.0)
    ones1b = sb.tile([128, 1], BF16, tag="ones1b")
    nc.vector.memset(ones1b[:], 1.0)
    # 16 stationary columns (all ones): dual-fp8 ldweights rejects very
    # narrow weights; the extra output partitions are duplicate sums.
    ones8 = sb.tile([128, KC, 16], FP8, tag="ones8")
    nc.vector.memset(ones8[:], 1.0)
    onesf = sb.tile([128, 1], F32, tag="onesf")
    nc.vector.memset(onesf[:], 1.0)
    neg1 = sb.tile([128, 1], F32, tag="neg1")
    nc.vector.memset(neg1[:], -1.0)
    neg2 = sb.tile([128, 1], F32, tag="neg2")
    nc.vector.memset(neg2[:], -2.0)

    # ---- persistent tiles ----
    z8 = sb.tile([128, KC, N], FP8, tag="z8")        # raw fp8 reps (D-major)
    zb = sb.tile([128, KC, N], BF16, tag="zb")       # raw bf16 reps
    z8n = sb.tile([128, KC, LOCAL], FP8, tag="z8n")  # normalized local cols
    # one PSUM bank for the small accumulators: cols 0:64 = ss_T (compact
    # col sums of squares), 64:72 = pos_T, 72 = psf.  Groups here are
    # strictly sequential in the PE stream.
    smalls = psr.tile([128, NT + 9], F32, tag="smalls")

    def ss_col(i, n=1):
        return smalls[:, bass.ds(i, n)]

    def pos_col(t):
        return smalls[:, bass.ds(NT + t, 1)]
    lns = sb.tile([128, NT], F32, tag="lns")
    inv_T = sb.tile([128, NT], F32, tag="inv_T")     # 1/|col|, compact
    s1_T = sb.tile([128, NT], F32, tag="s1_T")       # 2*A8*inv (schraudolph)
    s2_T = sb.tile([128, NT], F32, tag="s2_T")       # 2*inv (ACT exp scale)
    # row sums: [16, 1024]: h0 in bank A (cols 0:512), h1 in bank B, so
    # the two long-lived accumulation groups sit in different regions.
    # Partitions 1-15 hold duplicate sums (16-wide dual-fp8 stationary).
    rows = psr.tile([16, LOCAL], F32, tag="rows")
    fin = sb.tile([1, 3], F32, tag="fin")
    pc = sb.tile([128, 8], F32, tag="pc")
    posred = sb.tile([128, 1], F32, tag="posred")

    # ---- input DMAs (all SP; global DMA bw is the shared resource) ----
    def dma_zb(c0, n):
        nc.sync.dma_start(out=zb[:, :, bass.ds(c0, n)],
                          in_=zb_ap[:, :, bass.ds(c0, n)])

    def dma_z8(c0, n):
        nc.sync.dma_start(out=z8[:, :, bass.ds(c0, n)].bitcast(I8),
                          in_=z8_ap[:, :, bass.ds(c0, n)])

    dma_zb(0, 512)          # local head: gates the whole z8n chain
    dma_zb(512, 512)
    dma_zb(1024, SQC)
    dma_z8(0, 2048)         # stationary cols for sections 0-1
    dma_zb(2048, SQC)
    dma_zb(3072, SQC)
    dma_z8(2048, 2048)      # sections 2-3
    dma_zb(4096, SQC)
    dma_zb(5120, SQC)
    dma_z8(4096, 2048)      # sections 4-5
    dma_zb(6144, SQC)
    dma_zb(7168, SQC)
    dma_z8(6144, 2048)      # sections 6-7

    # Pin the ACT table set to natural_log_exp_and_others (id 6).
    nc.scalar.add_instruction(mybir.InstLoadActFuncSet(
        name=nc.get_next_instruction_name(), ins=[], outs=[],
        act_func_set_id=6))

    # ---- helpers ----
    sqs = {}

    def squares(c):
        # Pool owns squares (SBUF only); its slack absorbs the DMA pace.
        sq = sbsq.tile([128, KC, SQC], BF16, tag="sq")
        csl = bass.ds(c * SQC, SQC)
        nc.gpsimd.tensor_mul(sq[:], zb[:, :, csl], zb[:, :, csl])
        sqs[c] = sq

    def ss_chunk(c):
        sq = sqs[c]
        for t in range(SQC // 128):
            ct = c * (SQC // 128) + t
            for k in range(KC):
                nc.tensor.matmul(ss_col(ct),
                                 sq[:, k, bass.ds(t * 128, 128)], ones1b[:],
                                 start=(k == 0), stop=(k == KC - 1))

    def inv_chunk(c, n=1):
        # ACT Ln + Exp on [128, 8n]; Pool derives the two scale vectors
        sl = bass.ds(c * 8, 8 * n)
        nc.scalar.activation(lns[:, sl], ss_col(c * 8, 8 * n), AF.Ln,
                             bias=0.0, scale=1.0)
        nc.scalar.activation(inv_T[:, sl], lns[:, sl], AF.Exp,
                             bias=0.0, scale=-0.5)
        nc.gpsimd.tensor_scalar_mul(s1_T[:, sl], inv_T[:, sl], 2.0 * A8)
        nc.gpsimd.tensor_scalar_mul(s2_T[:, sl], inv_T[:, sl], 2.0)

    # ---- prologue: local normalization chain ----
    sq0 = sbsq.tile([128, KC, SQC], BF16, tag="sq")
    sqs[0] = sq0
    ib = sbt.tile([128, LOCAL], BF16, tag="invbl")
    for h in range(2):
        hsl = bass.ds(h * HALF, HALF)
        nc.gpsimd.tensor_mul(sq0[:, :, hsl], zb[:, :, hsl], zb[:, :, hsl])
        nb = ps.tile([128, HALF], F32, tag="pst")
        for k in range(KC):
            nc.tensor.matmul(nb[:], onesb[:], sq0[:, k, hsl],
                             start=(k == 0), stop=(k == KC - 1))
        # quake rsqrt: DVE copies PSUM->bf16 bits, then two 4x int ops
        ibt = sbt.tile([128, HALF], I16, tag="ibt")
        nc.vector.tensor_copy(ib[:, hsl], nb[:])
        nc.vector.tensor_scalar(ibt[:], ib[:, hsl].bitcast(I16), 1, -1,
                                ALU.logical_shift_right, ALU.bitwise_xor)
        nc.vector.tensor_scalar(ib[:, hsl].bitcast(I16), ibt[:], K16 + 1,
                                None, ALU.add)
        for k in range(KC):
            nc.gpsimd.tensor_mul(z8n[:, k, hsl], zb[:, k, hsl], ib[:, hsl])
    ss_chunk(0)
    inv_chunk(0)
    squares(1)
    ss_chunk(1)
    inv_chunk(1)
    prods = []

    # ---- main loop: one unit per column tile ----
    e8_cur = [None]

    def emit_unit(ct):
        par = ct % 2
        for h in range(2):
            eng = ASSIGN[2 * ct + h]
            pst = ps.tile([128, HALF], F32, tag="pst")
            nc.tensor.matmul(pst[:],
                             z8[:, :, bass.ds(ct * 128, 128)],
                             z8n[:, :, bass.ds(h * HALF, HALF)],
                             perf_mode=DR, start=True, stop=True)
            ev = e8_cur[0][:, par, bass.ds(h * HALF, HALF)]
            if eng == "A":
                nc.scalar.activation(ev, pst[:], AF.Exp, bias=neg2[:],
                                     scale=s2_T[:, bass.ds(ct, 1)])
            else:
                nc.vector.tensor_scalar(ev.bitcast(I8), pst[:],
                                        s1_T[:, bass.ds(ct, 1)], SC2,
                                        ALU.mult, ALU.add)

    SQ_AT = {4: 2, 10: 3, 24: 4, 30: 5, 40: 6, 41: 7}
    for c in range(NSQ):                      # 1024-col sections
        if c in (2, 4, 6):                    # paired inv to cut ACT ops
            ss_chunk(c)
            ss_chunk(c + 1)
            inv_chunk(c, 2)
        for ct in range(8 * c, 8 * c + 8):
            # squares paced to their zb DMA arrival; prods after sq4
            if ct in SQ_AT:
                squares(SQ_AT[ct])
            if ct == 26:
                for k in range(KC):
                    prod = sbt.tile([128, LOCAL], BF16, tag="prod")
                    nc.gpsimd.tensor_mul(prod[:], zb[:, k, bass.ds(0, LOCAL)],
                                         zb[:, k, bass.ds(B, LOCAL)])
                    prods.append(prod)
            if ct == 42:
                # positives: partition-sum of prods, then scale by invs
                for t in range(8):
                    for k in range(KC):
                        nc.tensor.matmul(pos_col(t),
                                         prods[k][:, bass.ds(t * 128, 128)],
                                         ones1b[:],
                                         start=(k == 0), stop=(k == KC - 1))
                nc.vector.tensor_mul(pc[:], smalls[:, bass.ds(NT, 8)],
                                     inv_T[:, bass.ds(0, 8)])
                nc.vector.tensor_mul(pc[:], pc[:], inv_T[:, bass.ds(32, 8)])
                nc.vector.reduce_sum(posred[:], pc[:], axis=X)
            if ct % 2 == 0:
                e8_cur[0] = sbe.tile([128, 2, LOCAL], FP8, name="e8t",
                                     tag="e8")
            emit_unit(ct)
            if ct % 2 == 1:
                cp = ct // 2
                for h in range(2):
                    nc.tensor.matmul(
                        rows[:, bass.ds(h * HALF, HALF)], ones8[:],
                        e8_cur[0][:, :, bass.ds(h * HALF, HALF)],
                        perf_mode=DR, start=(cp == 0), stop=(cp == 31))

    # ---- epilogue ----
    psf = smalls[0:1, bass.ds(NT + 8, 1)]
    nc.tensor.matmul(psf, onesf[:], posred[:], start=True, stop=True)
    nc.vector.tensor_copy(fin[0:1, bass.ds(2, 1)], psf)
    for h in range(2):
        lnr = sbt.tile([1, HALF], F32, tag="lnr")
        nc.scalar.activation(lnr[:], rows[0:1, bass.ds(h * HALF, HALF)],
                             AF.Ln, bias=neg1[0:1, :], scale=1.0,
                             accum_out=fin[0:1, bass.ds(h, 1)])
    nc.sync.dma_start(out=out_ap[:, :], in_=fin[:])


_NC_CACHE = None


def _build_program():
    global _NC_CACHE
    if _NC_CACHE is not None:
        return _NC_CACHE
    nc = bacc.Bacc("TRN2", target_bir_lowering=False, debug=False,
                   num_devices=N_CORES)
    z8 = nc.dram_tensor("z8", [128, KC, N], I8, kind="ExternalInput").ap()
    zb = nc.dram_tensor("zb", [128, KC, N], BF16, kind="ExternalInput").ap()
    out = nc.dram_tensor("out", [1, 3], F32, kind="ExternalOutput").ap()
    with tile.TileContext(nc) as tc:
        _ntxent_kernel(tc, z8, zb, out)
    nc.finalize()
    _NC_CACHE = nc
    return nc


def _prep_inputs(zis, zjs):
    """Host prep: stack, transpose, quantize, and roll per core."""
    zT = np.ascontiguousarray(
        np.concatenate([zjs, zis], axis=0).T.astype(np.float32, copy=False))
    zk = zT.reshape(KC, 128, N).transpose(1, 0, 2)       # [128, KC, N]
    z8 = zk.astype(E4)
    zbh = zk.astype(BF)
    in_maps = []
    for c in range(N_CORES):
        in_maps.append({
            "z8": np.ascontiguousarray(
                np.roll(z8, -c * LOCAL, axis=2)).view(np.int8),
            "zb": np.ascontiguousarray(np.roll(zbh, -c * LOCAL, axis=2)),
        })
    return in_maps


def kernel(zis: np.ndarray, zjs: np.ndarray) -> np.ndarray:
    assert zis.shape == (B, D) and zjs.shape == (B, D)
    nc = _build_program()
    in_maps = _prep_inputs(zis, zjs)
    res = run_bass_kernel_spmd(nc, in_maps, core_ids=list(range(N_CORES)))

    log_sum = 0.0
    pos_sum = 0.0
    for c in range(N_CORES):
        o = res.results[c]["out"]
        log_sum += float(o[0, 0]) + float(o[0, 1])
        pos_sum += float(o[0, 2])
    loss = 2.0 + (log_sum - 2.0 * pos_sum) / N
    return np.asarray(loss, dtype=np.float32)


# revision 26
# speedup vs baseline: 2.0300x; 1.0027x over previous
"""NT-Xent loss on 8 Trainium2 NeuronCores (Bass/Tile).

Math
----
reference: rows = interleave(zjs, zis) [2B, D]; zn = rows/max(|row|,eps);
S = (zn @ zn.T)/0.5; mask diag; loss = -mean_i log_softmax(S)[i, pair(i)].

The loss is invariant to any joint row/column permutation, so we use the
STACKED order rows = [zjs; zis] with pair(i) = i +- B.  Every score is
2*cos <= 2 and the diagonal exp(2*cos_ii - 2) == 1, so no masking or
row-max pass is needed:

    lse_i  = 2 + ln( sum_j exp(2 cos_ij - 2) - 1 )
    loss   = 2 + ( sum_i ln(rowsum_i - 1) - 2 * sum_i cos_{i,pair(i)} ) / 2B

Kernel strategy (per core; inputs rolled by c*1024 columns so columns
[0:1024] are the local row block, [4096:5120] the positives):

* similarity blocks are computed TRANSPOSED: one unit = [128 global
  cols x 1024 local rows], via fp8e4m3 DoubleRow matmuls (both k-tiles
  per instruction, 0.5 cyc/row).  The stationary side is RAW quantized
  z8; only the 1024 local (moving) columns are pre-normalized.  The
  missing 1/|col| factor rides into the exp as a per-partition scale.
* exp(2 cos - 2) must read PSUM, which only ACT and DVE can (GPSIMD
  cannot): ACT units use the hw Exp table; DVE units use a Schraudolph
  bit hack (t = x*8/ln2 + const, truncate to int8, bitcast fp8e4m3),
  tuned so the rowsum bias is ~1e-4.  All exp outputs are fp8.
* per-row sums of exp become PARTITION-axis sums on the Tensor engine:
  ones-stationary DoubleRow matmuls (16-wide stationary: dual-fp8
  ldweights rejects narrower weights) accumulate into a [16, 1024]
  PSUM tile, one bank per row-half (~107ns per 128x512 block).
* Pool (GPSIMD) owns all SBUF-side support: squares, the local
  normalize-multiply, positive-pair products and the scale vectors.
  Column norms: compact sq colsums via tiny matmuls -> ACT Ln/Exp on
  [128, 8]; the local broadcast invnorm uses a quake-style rsqrt bit
  hack on DVE (no PSUM->SBUF round trip through ACT).

Host-side work is layout + dtype quantization only; all arithmetic is
on device.  Per-core output is [1, 3]: two ln-rowsum partials and the
positive-cosine partial; the host sums 8x3 scalars.
"""

import numpy as np
import ml_dtypes
from contextlib import ExitStack

import concourse.bass as bass
import concourse.tile as tile
from concourse import bacc, mybir
from concourse.bass_utils import run_bass_kernel_spmd
from concourse._compat import with_exitstack

B = 4096
D = 256
N = 2 * B                  # 8192 rows/cols of the similarity matrix
N_CORES = 8
LOCAL = N // N_CORES       # 1024 local rows per core
KC = D // 128              # 2 contraction k-tiles
NT = N // 128              # 64 column tiles of 128
HALF = 512
SQC = 1024                 # squares chunk (columns)
NSQ = N // SQC             # 8 squares chunks
F32 = mybir.dt.float32
BF16 = mybir.dt.bfloat16
FP8 = mybir.dt.float8e4
I8 = mybir.dt.int8
I16 = mybir.dt.int16
AF = mybir.ActivationFunctionType
DR = mybir.MatmulPerfMode.DoubleRow
X = mybir.AxisListType.X
ALU = mybir.AluOpType

E4 = ml_dtypes.float8_e4m3
BF = ml_dtypes.bfloat16

# Schraudolph fp8e4m3 exp: bits = trunc(x * 8/ln2 + SC2); the -0.4 bias
# is tuned so the mean approx/exact ratio over the cos distribution ~ 1.
A8 = 8.0 / np.log(2.0)
SC2 = 56.0 - 2.0 * A8 + 0.5 - 0.4
K16 = 0x5F38   # quake-rsqrt magic for bf16 bits (tuned end-to-end)

# engine per half-tile: "A" (ACT exp) or "D" (DVE schraudolph).
QUOTA = {"A": 64, "D": 64}
D_FROM = 2     # first half-tile eligible for DVE


def _mk_assign():
    acc = {"A": 0.0, "D": 0.0}
    left = dict(QUOTA)
    out = []
    for t in range(2 * NT):
        for e in ("A", "D"):
            if left[e] > 0 and (e != "D" or t >= D_FROM):
                acc[e] += QUOTA[e] / (2 * NT if e == "A" else 2 * NT - D_FROM)
        cand = [e for e in ("A", "D") if left[e] > 0
                and (e != "D" or t >= D_FROM)]
        e = max(cand, key=lambda k: acc[k])
        acc[e] -= 1.0
        left[e] -= 1
        out.append(e)
    return out


ASSIGN = _mk_assign()


@with_exitstack
def _ntxent_kernel(ctx: ExitStack, tc: tile.TileContext, z8_ap, zb_ap, out_ap):
    nc = tc.nc

    sb = ctx.enter_context(tc.tile_pool(name="sb", bufs=1))
    sbsq = ctx.enter_context(tc.tile_pool(name="sq", bufs=NSQ))
    sbe = ctx.enter_context(tc.tile_pool(name="e8", bufs=8))
    sbt = ctx.enter_context(tc.tile_pool(name="tmp", bufs=2))
    ps = ctx.enter_context(tc.tile_pool(name="ps", bufs=5, space="PSUM"))
    psr = ctx.enter_context(tc.tile_pool(name="psr", bufs=1, space="PSUM"))

    # ---- constants ----
    onesb = sb.tile([128, 128], BF16, tag="onesb")
    nc.vector.memset(onesb[:], 1.0)
    ones1b = sb.tile([128, 1], BF16, tag="ones1b")
    nc.vector.memset(ones1b[:], 1.0)
    # 16 stationary columns (all ones): dual-fp8 ldweights rejects very
    # narrow weights; the extra output partitions are duplicate sums.
    ones8 = sb.tile([128, KC, 16], FP8, tag="ones8")
    nc.vector.memset(ones8[:], 1.0)
    onesf = sb.tile([128, 1], F32, tag="onesf")
    nc.vector.memset(onesf[:], 1.0)
    neg1 = sb.tile([128, 1], F32, tag="neg1")
    nc.vector.memset(neg1[:], -1.0)
    neg2 = sb.tile([128, 1], F32, tag="neg2")
    nc.vector.memset(neg2[:], -2.0)

    # ---- persistent tiles ----
    z8 = sb.tile([128, KC, N], FP8, tag="z8")        # raw fp8 reps (D-major)
    zb = sb.tile([128, KC, N], BF16, tag="zb")       # raw bf16 reps
    z8n = sb.tile([128, KC, LOCAL], FP8, tag="z8n")  # normalized local cols
    # one PSUM bank for the small accumulators: cols 0:64 = ss_T (compact
    # col sums of squares), 64:72 = pos_T, 72 = psf.  Groups here are
    # strictly sequential in the PE stream.
    smalls = psr.tile([128, NT + 9], F32, tag="smalls")

    def ss_col(i, n=1):
        return smalls[:, bass.ds(i, n)]

    def pos_col(t):
        return smalls[:, bass.ds(NT + t, 1)]
    lns = sb.tile([128, NT], F32, tag="lns")
    inv_T = sb.tile([128, NT], F32, tag="inv_T")     # 1/|col|, compact
    s1_T = sb.tile([128, NT], F32, tag="s1_T")       # 2*A8*inv (schraudolph)
    s2_T = sb.tile([128, NT], F32, tag="s2_T")       # 2*inv (ACT exp scale)
    # row sums: [16, 1024]: h0 in bank A (cols 0:512), h1 in bank B, so
    # the two long-lived accumulation groups sit in different regions.
    # Partitions 1-15 hold duplicate sums (16-wide dual-fp8 stationary).
    rows = psr.tile([16, LOCAL], F32, tag="rows")
    fin = sb.tile([1, 3], F32, tag="fin")
    pc = sb.tile([128, 8], F32, tag="pc")
    posred = sb.tile([128, 1], F32, tag="posred")

    # ---- input DMAs (all SP; global DMA bw is the shared resource) ----
    def dma_zb(c0, n):
        nc.sync.dma_start(out=zb[:, :, bass.ds(c0, n)],
                          in_=zb_ap[:, :, bass.ds(c0, n)])

    def dma_z8(c0, n):
        nc.sync.dma_start(out=z8[:, :, bass.ds(c0, n)].bitcast(I8),
                          in_=z8_ap[:, :, bass.ds(c0, n)])

    dma_zb(0, 512)          # local head: gates the whole z8n chain
    dma_zb(512, 512)
    dma_zb(1024, SQC)
    dma_z8(0, 2048)         # stationary cols for sections 0-1
    dma_zb(2048, SQC)
    dma_zb(3072, SQC)
    dma_z8(2048, 2048)      # sections 2-3
    dma_zb(4096, SQC)
    dma_zb(5120, SQC)
    dma_z8(4096, 2048)      # sections 4-5
    dma_zb(6144, SQC)
    dma_zb(7168, SQC)
    dma_z8(6144, 2048)      # sections 6-7

    # Pin the ACT table set to natural_log_exp_and_others (id 6).
    nc.scalar.add_instruction(mybir.InstLoadActFuncSet(
        name=nc.get_next_instruction_name(), ins=[], outs=[],
        act_func_set_id=6))

    # ---- helpers ----
    sqs = {}

    def squares(c):
        # Pool owns squares (SBUF only); its slack absorbs the DMA pace.
        sq = sbsq.tile([128, KC, SQC], BF16, tag="sq")
        csl = bass.ds(c * SQC, SQC)
        nc.gpsimd.tensor_mul(sq[:], zb[:, :, csl], zb[:, :, csl])
        sqs[c] = sq

    def ss_chunk(c):
        sq = sqs[c]
        for t in range(SQC // 128):
            ct = c * (SQC // 128) + t
            for k in range(KC):
                nc.tensor.matmul(ss_col(ct),
                                 sq[:, k, bass.ds(t * 128, 128)], ones1b[:],
                                 start=(k == 0), stop=(k == KC - 1))

    def inv_chunk(c, n=1):
        # ACT Ln + Exp on [128, 8n]; Pool derives the two scale vectors
        sl = bass.ds(c * 8, 8 * n)
        nc.scalar.activation(lns[:, sl], ss_col(c * 8, 8 * n), AF.Ln,
                             bias=0.0, scale=1.0)
        nc.scalar.activation(inv_T[:, sl], lns[:, sl], AF.Exp,
                             bias=0.0, scale=-0.5)
        nc.gpsimd.tensor_scalar_mul(s1_T[:, sl], inv_T[:, sl], 2.0 * A8)
        nc.gpsimd.tensor_scalar_mul(s2_T[:, sl], inv_T[:, sl], 2.0)

    # ---- prologue: local normalization chain ----
    sq0 = sbsq.tile([128, KC, SQC], BF16, tag="sq")
    sqs[0] = sq0
    ib = sbt.tile([128, LOCAL], BF16, tag="invbl")
    for h in range(2):
        hsl = bass.ds(h * HALF, HALF)
        # DVE for the gating chunk: 2x mode and fewer engine handoffs
        nc.vector.tensor_mul(sq0[:, :, hsl], zb[:, :, hsl], zb[:, :, hsl])
        nb = ps.tile([128, HALF], F32, tag="pst")
        for k in range(KC):
            nc.tensor.matmul(nb[:], onesb[:], sq0[:, k, hsl],
                             start=(k == 0), stop=(k == KC - 1))
        # quake rsqrt: DVE copies PSUM->bf16 bits, then two 4x int ops
        ibt = sbt.tile([128, HALF], I16, tag="ibt")
        nc.vector.tensor_copy(ib[:, hsl], nb[:])
        nc.vector.tensor_scalar(ibt[:], ib[:, hsl].bitcast(I16), 1, -1,
                                ALU.logical_shift_right, ALU.bitwise_xor)
        nc.vector.tensor_scalar(ib[:, hsl].bitcast(I16), ibt[:], K16 + 1,
                                None, ALU.add)
        for k in range(KC):
            # h0 applies on DVE: they gate the first matmuls and Pool's
            # greedy scheduler would run sq1 ahead of them
            eng = nc.vector if h == 0 else nc.gpsimd
            eng.tensor_mul(z8n[:, k, hsl], zb[:, k, hsl], ib[:, hsl])
    ss_chunk(0)
    inv_chunk(0)
    prods = []

    # ---- main loop: one unit per column tile ----
    e8_cur = [None]

    def emit_unit(ct):
        par = ct % 2
        for h in range(2):
            eng = ASSIGN[2 * ct + h]
            pst = ps.tile([128, HALF], F32, tag="pst")
            nc.tensor.matmul(pst[:],
                             z8[:, :, bass.ds(ct * 128, 128)],
                             z8n[:, :, bass.ds(h * HALF, HALF)],
                             perf_mode=DR, start=True, stop=True)
            ev = e8_cur[0][:, par, bass.ds(h * HALF, HALF)]
            if eng == "A":
                nc.scalar.activation(ev, pst[:], AF.Exp, bias=neg2[:],
                                     scale=s2_T[:, bass.ds(ct, 1)])
            else:
                nc.vector.tensor_scalar(ev.bitcast(I8), pst[:],
                                        s1_T[:, bass.ds(ct, 1)], SC2,
                                        ALU.mult, ALU.add)

    SQ_AT = {0: 1, 4: 2, 10: 3, 24: 4, 30: 5, 40: 6, 41: 7}
    for c in range(NSQ):                      # 1024-col sections
        if c == 1:
            ss_chunk(1)
            inv_chunk(1)
        elif c in (2, 4, 6):                  # paired inv to cut ACT ops
            ss_chunk(c)
            ss_chunk(c + 1)
            inv_chunk(c, 2)
        for ct in range(8 * c, 8 * c + 8):
            # squares paced to their zb DMA arrival; prods after sq4
            if ct in SQ_AT:
                squares(SQ_AT[ct])
            if ct == 26:
                for k in range(KC):
                    prod = sbt.tile([128, LOCAL], BF16, tag="prod")
                    nc.gpsimd.tensor_mul(prod[:], zb[:, k, bass.ds(0, LOCAL)],
                                         zb[:, k, bass.ds(B, LOCAL)])
                    prods.append(prod)
            if ct == 42:
                # positives: partition-sum of prods, then scale by invs
                for t in range(8):
                    for k in range(KC):
                        nc.tensor.matmul(pos_col(t),
                                         prods[k][:, bass.ds(t * 128, 128)],
                                         ones1b[:],
                                         start=(k == 0), stop=(k == KC - 1))
                nc.vector.tensor_mul(pc[:], smalls[:, bass.ds(NT, 8)],
                                     inv_T[:, bass.ds(0, 8)])
                nc.vector.tensor_mul(pc[:], pc[:], inv_T[:, bass.ds(32, 8)])
                nc.vector.reduce_sum(posred[:], pc[:], axis=X)
            if ct % 2 == 0:
                e8_cur[0] = sbe.tile([128, 2, LOCAL], FP8, name="e8t",
                                     tag="e8")
            emit_unit(ct)
            if ct % 2 == 1:
                cp = ct // 2
                for h in range(2):
                    nc.tensor.matmul(
                        rows[:, bass.ds(h * HALF, HALF)], ones8[:],
                        e8_cur[0][:, :, bass.ds(h * HALF, HALF)],
                        perf_mode=DR, start=(cp == 0), stop=(cp == 31))

    # ---- epilogue ----
    psf = smalls[0:1, bass.ds(NT + 8, 1)]
    nc.tensor.matmul(psf, onesf[:], posred[:], start=True, stop=True)
    nc.vector.tensor_copy(fin[0:1, bass.ds(2, 1)], psf)
    for h in range(2):
        lnr = sbt.tile([1, HALF], F32, tag="lnr")
        nc.scalar.activation(lnr[:], rows[0:1, bass.ds(h * HALF, HALF)],
                             AF.Ln, bias=neg1[0:1, :], scale=1.0,
                             accum_out=fin[0:1, bass.ds(h, 1)])
    nc.sync.dma_start(out=out_ap[:, :], in_=fin[:])


_NC_CACHE = None


def _build_program():
    global _NC_CACHE
    if _NC_CACHE is not None:
        return _NC_CACHE
    nc = bacc.Bacc("TRN2", target_bir_lowering=False, debug=False,
                   num_devices=N_CORES)
    z8 = nc.dram_tensor("z8", [128, KC, N], I8, kind="ExternalInput").ap()
    zb = nc.dram_tensor("zb", [128, KC, N], BF16, kind="ExternalInput").ap()
    out = nc.dram_tensor("out", [1, 3], F32, kind="ExternalOutput").ap()
    with tile.TileContext(nc) as tc:
        _ntxent_kernel(tc, z8, zb, out)
    nc.finalize()
    _NC_CACHE = nc
    return nc


def _prep_inputs(zis, zjs):
    """Host prep: stack, transpose, quantize, and roll per core."""
    zT = np.ascontiguousarray(
        np.concatenate([zjs, zis], axis=0).T.astype(np.float32, copy=False))
    zk = zT.reshape(KC, 128, N).transpose(1, 0, 2)       # [128, KC, N]
    z8 = zk.astype(E4)
    zbh = zk.astype(BF)
    in_maps = []
    for c in range(N_CORES):
        in_maps.append({
            "z8": np.ascontiguousarray(
                np.roll(z8, -c * LOCAL, axis=2)).view(np.int8),
            "zb": np.ascontiguousarray(np.roll(zbh, -c * LOCAL, axis=2)),
        })
    return in_maps


def kernel(zis: np.ndarray, zjs: np.ndarray) -> np.ndarray:
    assert zis.shape == (B, D) and zjs.shape == (B, D)
    nc = _build_program()
    in_maps = _prep_inputs(zis, zjs)
    res = run_bass_kernel_spmd(nc, in_maps, core_ids=list(range(N_CORES)))

    log_sum = 0.0
    pos_sum = 0.0
    for c in range(N_CORES):
        o = res.results[c]["out"]
        log_sum += float(o[0, 0]) + float(o[0, 1])
        pos_sum += float(o[0, 2])
    loss = 2.0 + (log_sum - 2.0 * pos_sum) / N
    return np.asarray(loss, dtype=np.float32)


# revision 34
# speedup vs baseline: 2.1440x; 1.0562x over previous
"""NT-Xent loss on 8 Trainium2 NeuronCores (Bass/Tile).

Math
----
reference: rows = interleave(zjs, zis) [2B, D]; zn = rows/max(|row|,eps);
S = (zn @ zn.T)/0.5; mask diag; loss = -mean_i log_softmax(S)[i, pair(i)].

The loss is invariant to any joint row/column permutation, so we use the
STACKED order rows = [zjs; zis] with pair(i) = i +- B.  Every score is
2*cos <= 2 and the diagonal exp(2*cos_ii - 2) == 1, so no masking or
row-max pass is needed:

    lse_i  = 2 + ln( sum_j exp(2 cos_ij - 2) - 1 )
    loss   = 2 + ( sum_i ln(rowsum_i - 1) - 2 * sum_i cos_{i,pair(i)} ) / 2B

Kernel strategy (per core; inputs rolled by c*1024 columns so columns
[0:1024] are the local row block, [4096:5120] the positives):

* similarity blocks are computed TRANSPOSED: one unit = [128 global
  cols x 1024 local rows], via fp8e4m3 DoubleRow matmuls (both k-tiles
  per instruction, 0.5 cyc/row).  The stationary side is RAW quantized
  z8; only the 1024 local (moving) columns are pre-normalized.  The
  missing 1/|col| factor rides into the exp as a per-partition scale.
* exp(2 cos - 2) must read PSUM, which only ACT and DVE can (GPSIMD
  cannot): ACT units use the hw Exp table; DVE units use a Schraudolph
  bit hack (t = x*8/ln2 + const, truncate to int8, bitcast fp8e4m3),
  tuned so the rowsum bias is ~1e-4.  All exp outputs are fp8.
* per-row sums of exp become PARTITION-axis sums on the Tensor engine:
  ones-stationary DoubleRow matmuls (16-wide stationary: dual-fp8
  ldweights rejects narrower weights) accumulate into a [16, 1024]
  PSUM tile, one bank per row-half (~107ns per 128x512 block).
* Pool (GPSIMD) owns all SBUF-side support: squares, the local
  normalize-multiply, positive-pair products and the scale vectors.
  Column norms: compact sq colsums via tiny matmuls -> ACT Ln/Exp on
  [128, 8]; the local broadcast invnorm uses a quake-style rsqrt bit
  hack on DVE (no PSUM->SBUF round trip through ACT).

Host-side work is layout + dtype quantization only; all arithmetic is
on device.  Per-core output is [1, 3]: two ln-rowsum partials and the
positive-cosine partial; the host sums 8x3 scalars.
"""

import numpy as np
import ml_dtypes
from contextlib import ExitStack

import concourse.bass as bass
import concourse.tile as tile
from concourse import bacc, mybir
from concourse.bass_utils import run_bass_kernel_spmd
from concourse._compat import with_exitstack

B = 4096
D = 256
N = 2 * B                  # 8192 rows/cols of the similarity matrix
N_CORES = 8
LOCAL = N // N_CORES       # 1024 local rows per core
KC = D // 128              # 2 contraction k-tiles
NT = N // 128              # 64 column tiles of 128
HALF = 512
SQC = 1024                 # squares chunk (columns)
NSQ = N // SQC             # 8 squares chunks
F32 = mybir.dt.float32
BF16 = mybir.dt.bfloat16
FP8 = mybir.dt.float8e4
I8 = mybir.dt.int8
I16 = mybir.dt.int16
AF = mybir.ActivationFunctionType
DR = mybir.MatmulPerfMode.DoubleRow
X = mybir.AxisListType.X
ALU = mybir.AluOpType

E4 = ml_dtypes.float8_e4m3
BF = ml_dtypes.bfloat16

# Schraudolph fp8e4m3 exp: bits = trunc(x * 8/ln2 + SC2); the -0.4 bias
# is tuned so the mean approx/exact ratio over the cos distribution ~ 1.
A8 = 8.0 / np.log(2.0)
SC2 = 56.0 - 2.0 * A8 + 0.5 - 0.4
K16 = 0x5F38   # quake-rsqrt magic for bf16 bits (tuned end-to-end)

# engine per col-tile: "A" (ACT exp on a [128,1024] PSUM pair) or "D"
# (DVE schraudolph on two [128,512] singles).
QUOTA = {"A": 36, "D": 28}
D_FROM = 1     # first col-tile eligible for DVE


def _mk_assign():
    acc = {"A": 0.0, "D": 0.0}
    left = dict(QUOTA)
    out = []
    for t in range(NT):
        for e in ("A", "D"):
            if left[e] > 0 and (e != "D" or t >= D_FROM):
                acc[e] += QUOTA[e] / (NT if e == "A" else NT - D_FROM)
        cand = [e for e in ("A", "D") if left[e] > 0
                and (e != "D" or t >= D_FROM)]
        e = max(cand, key=lambda k: acc[k])
        acc[e] -= 1.0
        left[e] -= 1
        out.append(e)
    return out


ASSIGN = _mk_assign()


@with_exitstack
def _ntxent_kernel(ctx: ExitStack, tc: tile.TileContext, z8_ap, zb_ap, out_ap):
    nc = tc.nc

    sb = ctx.enter_context(tc.tile_pool(name="sb", bufs=1))
    sbsq = ctx.enter_context(tc.tile_pool(name="sq", bufs=NSQ))
    sbe = ctx.enter_context(tc.tile_pool(name="e8", bufs=8))
    sbt = ctx.enter_context(tc.tile_pool(name="tmp", bufs=2))
    ps = ctx.enter_context(tc.tile_pool(name="ps", bufs=2, space="PSUM"))
    psa = ctx.enter_context(tc.tile_pool(name="psa", bufs=2, space="PSUM"))
    psr = ctx.enter_context(tc.tile_pool(name="psr", bufs=1, space="PSUM"))

    # ---- constants ----
    onesb = sb.tile([128, 128], BF16, tag="onesb")
    nc.vector.memset(onesb[:], 1.0)
    ones1b = sb.tile([128, 1], BF16, tag="ones1b")
    nc.vector.memset(ones1b[:], 1.0)
    # Half-masked dual-fp8 ones stationaries: h0 sums land on output
    # partitions 0-15, h1 on 16-31, so BOTH row-halves accumulate in ONE
    # psum bank as a single group (zero columns add zero elsewhere).
    ones8h = []
    for h in range(2):
        o = sb.tile([128, KC, 64], FP8, name="ones8", tag=f"ones8{h}")
        nc.vector.memset(o[:], 0.0)
        nc.vector.memset(o[:, :, bass.ds(32 * h, 16)], 1.0)
        ones8h.append(o)
    onesf = sb.tile([128, 1], F32, tag="onesf")
    nc.vector.memset(onesf[:], 1.0)
    neg1 = sb.tile([128, 1], F32, tag="neg1")
    nc.vector.memset(neg1[:], -1.0)
    neg2 = sb.tile([128, 1], F32, tag="neg2")
    nc.vector.memset(neg2[:], -2.0)

    # ---- persistent tiles ----
    z8 = sb.tile([128, KC, N], FP8, tag="z8")        # raw fp8 reps (D-major)
    zb = sb.tile([128, KC, N], BF16, tag="zb")       # raw bf16 reps
    z8n = sb.tile([128, KC, LOCAL], FP8, tag="z8n")  # normalized local cols
    # one PSUM bank for the small accumulators: cols 0:64 = ss_T (compact
    # col sums of squares), 64:72 = pos_T, 72 = psf.  Groups here are
    # strictly sequential in the PE stream.
    smalls = psr.tile([128, NT + 9], F32, tag="smalls")

    def ss_col(i, n=1):
        return smalls[:, bass.ds(i, n)]

    def pos_col(t):
        return smalls[:, bass.ds(NT + t, 1)]
    lns = sb.tile([128, NT], F32, tag="lns")
    inv_T = sb.tile([128, NT], F32, tag="inv_T")     # 1/|col|, compact
    s1_T = sb.tile([128, NT], F32, tag="s1_T")       # 2*A8*inv (schraudolph)
    s2_T = sb.tile([128, NT], F32, tag="s2_T")       # 2*inv (ACT exp scale)
    # row sums: ONE bank, ONE accumulation group: h0 rows on partitions
    # 0-15 (dup x16), h1 rows on partitions 32-47 (engine APs must start
    # at partition 0/32/64/96).
    rows = psr.tile([64, HALF], F32, tag="rows")
    fin = sb.tile([1, 3], F32, tag="fin")
    pc = sb.tile([128, 8], F32, tag="pc")
    posred = sb.tile([128, 1], F32, tag="posred")

    # ---- input DMAs (all SP; global DMA bw is the shared resource) ----
    def dma_zb(c0, n):
        nc.sync.dma_start(out=zb[:, :, bass.ds(c0, n)],
                          in_=zb_ap[:, :, bass.ds(c0, n)])

    def dma_z8(c0, n):
        nc.sync.dma_start(out=z8[:, :, bass.ds(c0, n)].bitcast(I8),
                          in_=z8_ap[:, :, bass.ds(c0, n)])

    dma_zb(0, 512)          # local head: gates the whole z8n chain
    dma_zb(512, 512)
    dma_zb(1024, SQC)
    dma_z8(0, 2048)         # stationary cols for sections 0-1
    dma_zb(2048, SQC)
    dma_zb(3072, SQC)
    dma_z8(2048, 2048)      # sections 2-3
    dma_zb(4096, SQC)
    dma_zb(5120, SQC)
    dma_z8(4096, 2048)      # sections 4-5
    dma_zb(6144, SQC)
    dma_zb(7168, SQC)
    dma_z8(6144, 2048)      # sections 6-7

    # Pin the ACT table set to natural_log_exp_and_others (id 6).
    nc.scalar.add_instruction(mybir.InstLoadActFuncSet(
        name=nc.get_next_instruction_name(), ins=[], outs=[],
        act_func_set_id=6))

    # ---- helpers ----
    sqs = {}

    def squares(c):
        # Pool owns squares (SBUF only); its slack absorbs the DMA pace.
        sq = sbsq.tile([128, KC, SQC], BF16, tag="sq")
        csl = bass.ds(c * SQC, SQC)
        nc.gpsimd.tensor_mul(sq[:], zb[:, :, csl], zb[:, :, csl])
        sqs[c] = sq

    def ss_chunk(c):
        sq = sqs[c]
        for t in range(SQC // 128):
            ct = c * (SQC // 128) + t
            for k in range(KC):
                nc.tensor.matmul(ss_col(ct),
                                 sq[:, k, bass.ds(t * 128, 128)], ones1b[:],
                                 start=(k == 0), stop=(k == KC - 1))

    def inv_chunk(c, n=1):
        # ACT Ln + Exp on [128, 8n]; Pool derives the two scale vectors
        sl = bass.ds(c * 8, 8 * n)
        nc.scalar.activation(lns[:, sl], ss_col(c * 8, 8 * n), AF.Ln,
                             bias=0.0, scale=1.0)
        nc.scalar.activation(inv_T[:, sl], lns[:, sl], AF.Exp,
                             bias=0.0, scale=-0.5)
        nc.gpsimd.tensor_scalar_mul(s1_T[:, sl], inv_T[:, sl], 2.0 * A8)
        nc.gpsimd.tensor_scalar_mul(s2_T[:, sl], inv_T[:, sl], 2.0)

    # ---- prologue: local normalization chain ----
    sq0 = sbsq.tile([128, KC, SQC], BF16, tag="sq")
    sqs[0] = sq0
    ib = sbt.tile([128, LOCAL], BF16, tag="invbl")
    for h in range(2):
        hsl = bass.ds(h * HALF, HALF)
        # DVE for the gating chunk: 2x mode and fewer engine handoffs
        nc.vector.tensor_mul(sq0[:, :, hsl], zb[:, :, hsl], zb[:, :, hsl])
        nb = ps.tile([128, HALF], F32, tag="pst")
        for k in range(KC):
            nc.tensor.matmul(nb[:], onesb[:], sq0[:, k, hsl],
                             start=(k == 0), stop=(k == KC - 1))
        # quake rsqrt: DVE copies PSUM->bf16 bits, then two 4x int ops
        ibt = sbt.tile([128, HALF], I16, tag="ibt")
        nc.vector.tensor_copy(ib[:, hsl], nb[:])
        nc.vector.tensor_scalar(ibt[:], ib[:, hsl].bitcast(I16), 1, -1,
                                ALU.logical_shift_right, ALU.bitwise_xor)
        nc.vector.tensor_scalar(ib[:, hsl].bitcast(I16), ibt[:], K16 + 1,
                                None, ALU.add)
        for k in range(KC):
            # h0 applies on DVE: they gate the first matmuls and Pool's
            # greedy scheduler would run sq1 ahead of them
            eng = nc.vector if h == 0 else nc.gpsimd
            eng.tensor_mul(z8n[:, k, hsl], zb[:, k, hsl], ib[:, hsl])
    ss_chunk(0)
    inv_chunk(0)
    prods = []

    # ---- main loop: one unit per column tile ----
    e8_cur = [None]

    def emit_unit(ct):
        par = ct % 2
        if ASSIGN[ct] == "A":
            pst = psa.tile([128, LOCAL], F32, name="psta", tag="psta")
            for h in range(2):
                nc.tensor.matmul(pst[:, bass.ds(h * HALF, HALF)],
                                 z8[:, :, bass.ds(ct * 128, 128)],
                                 z8n[:, :, bass.ds(h * HALF, HALF)],
                                 perf_mode=DR, start=True, stop=True)
            nc.scalar.activation(e8_cur[0][:, par, :], pst[:], AF.Exp,
                                 bias=neg2[:], scale=s2_T[:, bass.ds(ct, 1)])
        else:
            for h in range(2):
                pst = ps.tile([128, HALF], F32, tag="pst")
                nc.tensor.matmul(pst[:],
                                 z8[:, :, bass.ds(ct * 128, 128)],
                                 z8n[:, :, bass.ds(h * HALF, HALF)],
                                 perf_mode=DR, start=True, stop=True)
                ev = e8_cur[0][:, par, bass.ds(h * HALF, HALF)]
                nc.vector.tensor_scalar(ev.bitcast(I8), pst[:],
                                        s1_T[:, bass.ds(ct, 1)], SC2,
                                        ALU.mult, ALU.add)

    SQ_AT = {0: 1, 2: 2, 5: 3, 12: 4, 15: 5, 20: 6, 22: 7}
    for c in range(NSQ):                      # 1024-col sections
        if c == 1:
            ss_chunk(1)
            inv_chunk(1)
        elif c in (2, 4, 6):                  # paired inv to cut ACT ops
            ss_chunk(c)
            ss_chunk(c + 1)
            inv_chunk(c, 2)
        for ct in range(8 * c, 8 * c + 8):
            # squares paced to their zb DMA arrival; prods after sq4
            if ct in SQ_AT:
                squares(SQ_AT[ct])
            if ct == 13:
                for k in range(KC):
                    prod = sbt.tile([128, LOCAL], BF16, tag="prod")
                    nc.gpsimd.tensor_mul(prod[:], zb[:, k, bass.ds(0, LOCAL)],
                                         zb[:, k, bass.ds(B, LOCAL)])
                    prods.append(prod)
            if ct == 34:
                # positives: partition-sum of prods, then scale by invs
                for t in range(8):
                    for k in range(KC):
                        nc.tensor.matmul(pos_col(t),
                                         prods[k][:, bass.ds(t * 128, 128)],
                                         ones1b[:],
                                         start=(k == 0), stop=(k == KC - 1))
                nc.vector.tensor_mul(pc[:], smalls[:, bass.ds(NT, 8)],
                                     inv_T[:, bass.ds(0, 8)])
                nc.vector.tensor_mul(pc[:], pc[:], inv_T[:, bass.ds(32, 8)])
                nc.vector.reduce_sum(posred[:], pc[:], axis=X)
            if ct % 2 == 0:
                e8_cur[0] = sbe.tile([128, 2, LOCAL], FP8, name="e8t",
                                     tag="e8")
            emit_unit(ct)
            if ct % 2 == 1:
                cp = ct // 2
                for h in range(2):
                    nc.tensor.matmul(
                        rows[:, :], ones8h[h],
                        e8_cur[0][:, :, bass.ds(h * HALF, HALF)],
                        perf_mode=DR, start=(cp == 0 and h == 0),
                        stop=(cp == 31 and h == 1))

    # ---- epilogue ----
    psf = smalls[0:1, bass.ds(NT + 8, 1)]
    nc.tensor.matmul(psf, onesf[:], posred[:], start=True, stop=True)
    nc.vector.tensor_copy(fin[0:1, bass.ds(2, 1)], psf)
    for h in range(2):
        lnr = sbt.tile([1, HALF], F32, tag="lnr")
        nc.scalar.activation(lnr[:], rows[bass.ds(32 * h, 1), :],
                             AF.Ln, bias=neg1[0:1, :], scale=1.0,
                             accum_out=fin[0:1, bass.ds(h, 1)])
    nc.sync.dma_start(out=out_ap[:, :], in_=fin[:])


_NC_CACHE = None


def _build_program():
    global _NC_CACHE
    if _NC_CACHE is not None:
        return _NC_CACHE
    nc = bacc.Bacc("TRN2", target_bir_lowering=False, debug=False,
                   num_devices=N_CORES)
    z8 = nc.dram_tensor("z8", [128, KC, N], I8, kind="ExternalInput").ap()
    zb = nc.dram_tensor("zb", [128, KC, N], BF16, kind="ExternalInput").ap()
    out = nc.dram_tensor("out", [1, 3], F32, kind="ExternalOutput").ap()
    with tile.TileContext(nc) as tc:
        _ntxent_kernel(tc, z8, zb, out)
    nc.finalize()
    _NC_CACHE = nc
    return nc


def _prep_inputs(zis, zjs):
    """Host prep: stack, transpose, quantize, and roll per core."""
    zT = np.ascontiguousarray(
        np.concatenate([zjs, zis], axis=0).T.astype(np.float32, copy=False))
    zk = zT.reshape(KC, 128, N).transpose(1, 0, 2)       # [128, KC, N]
    z8 = zk.astype(E4)
    zbh = zk.astype(BF)
    in_maps = []
    for c in range(N_CORES):
        in_maps.append({
            "z8": np.ascontiguousarray(
                np.roll(z8, -c * LOCAL, axis=2)).view(np.int8),
            "zb": np.ascontiguousarray(np.roll(zbh, -c * LOCAL, axis=2)),
        })
    return in_maps


def kernel(zis: np.ndarray, zjs: np.ndarray) -> np.ndarray:
    assert zis.shape == (B, D) and zjs.shape == (B, D)
    nc = _build_program()
    in_maps = _prep_inputs(zis, zjs)
    res = run_bass_kernel_spmd(nc, in_maps, core_ids=list(range(N_CORES)))

    log_sum = 0.0
    pos_sum = 0.0
    for c in range(N_CORES):
        o = res.results[c]["out"]
        log_sum += float(o[0, 0]) + float(o[0, 1])
        pos_sum += float(o[0, 2])
    loss = 2.0 + (log_sum - 2.0 * pos_sum) / N
    return np.asarray(loss, dtype=np.float32)
